# revision 13
# baseline (speedup 1.0000x reference)
"""Bass/Tile TRN2 kernel for GroupedQueryAttention (B=2, T=2048, D=2048,
32 Q heads / 8 KV heads, hd=64, RoPE, causal), sharded over 8 NeuronCores
by KV head (1 KV head + 4 Q heads per core; wo row-sharded, partials
summed on host).

v2: single merged pipeline. fp8 DoubleRow projections (x and wq/wk/wv in
fp8e4m3, host-scaled), RoPE pair-swap via DVE stream_shuffle (sign baked
into the sin table), causal mask added in PSUM by an extra accumulating
matmul, bf16 attention (scores K=64 head-pairs run row-tiled/concurrent),
out-projection and projection matmuls woven into the attention i-loop as
fill work so the PE never idles (HAM throttle avoidance)."""

import sys

for _p in ("/opt/trn_rl_repo",):
    if _p not in sys.path:
        sys.path.insert(0, _p)

from collections import deque

import numpy as np
import ml_dtypes

import concourse.bass as bass
import concourse.mybir as mybir
import concourse.tile as tile
from concourse import bacc
from concourse.bass_utils import run_bass_kernel_spmd

F32 = mybir.dt.float32
BF16 = mybir.dt.bfloat16
FP8 = mybir.dt.float8e4
NPFP8 = mybir.dt.np(FP8)
NPBF16 = ml_dtypes.bfloat16
P = 128
HD = 64          # head dim
CH = 512         # token chunk (attention q-chunk and proj chunk)
NCORES = 8
DR = mybir.MatmulPerfMode.DoubleRow

SX = 4.0         # fp8 scale on x
SW = 32.0        # fp8 scale on wq/wk rows
SWV = 4.0        # fp8 scale on wv rows -> v_sb = 16*v, fixed via wo/16


def build_program(B=2, T=2048, D=2048):
    BT = B * T
    KT = D // P            # contraction tiles for projections (16)
    NCH = BT // CH         # chunks over all batches (8)
    NJ = T // CH           # q chunks per batch (4)
    NI = T // P            # k tiles per batch (16)

    nc = bacc.Bacc(None, target_bir_lowering=False, debug=False)

    xT_d = nc.dram_tensor("xT", [D, BT], BF16, kind="ExternalInput")
    wq_d = nc.dram_tensor("wqT", [D, 256], BF16, kind="ExternalInput")
    wkv_d = nc.dram_tensor("wkvT", [D, 128], BF16, kind="ExternalInput")
    wo_d = nc.dram_tensor("woT", [256, D], BF16, kind="ExternalInput")
    cs_d = nc.dram_tensor("cs", [P, T], BF16, kind="ExternalInput")
    sn_d = nc.dram_tensor("sn", [P, T], BF16, kind="ExternalInput")
    dmask_d = nc.dram_tensor("dmask", [P, P], F32, kind="ExternalInput")
    id64_d = nc.dram_tensor("id64", [HD, HD], F32, kind="ExternalInput")
    out_d = nc.dram_tensor("out", [BT, D], BF16, kind="ExternalOutput")

    SWAP = [(i ^ 1) for i in range(32)]

    with tile.TileContext(nc) as tc:
        with tc.tile_pool(name="persist", bufs=1) as persist:
            wq_sb = persist.tile([P, KT, 256], BF16, tag="wq")
            wkv_sb = persist.tile([P, KT, 128], BF16, tag="wkv")
            wo_sb = persist.tile([P, 2, D], BF16, tag="wo")
            cs_sb = persist.tile([P, T], BF16, tag="cs")
            sn_sb = persist.tile([P, T], BF16, tag="sn")
            dmask_sb = persist.tile([P, 2, P], F32, tag="dmask")
            id64_sb = persist.tile([HD, HD], F32, tag="id64")
            q_sb = persist.tile([P, 2, BT], BF16, tag="qcache")
            k_sb = persist.tile([P, B, T], BF16, tag="kcache")
            v_sb = persist.tile([P, B * NI, HD + 1], BF16, tag="vcache")

            nc.sync.dma_start(wq_sb[:], wq_d[:].rearrange("(ko p) m -> p ko m", p=P))
            nc.sync.dma_start(wkv_sb[:], wkv_d[:].rearrange("(ko p) m -> p ko m", p=P))
            nc.sync.dma_start(wo_sb[:], wo_d[:].rearrange("(fo p) n -> p fo n", p=P))
            nc.sync.dma_start(cs_sb[:], cs_d[:])
            nc.sync.dma_start(sn_sb[:], sn_d[:])
            nc.sync.dma_start(dmask_sb[:, 0, :], dmask_d[:])
            nc.sync.dma_start(dmask_sb[:, 1, :], dmask_d[:])
            nc.sync.dma_start(id64_sb[:], id64_d[:])
            nc.vector.memset(v_sb[:, :, HD:HD + 1], 1.0)

            with (
                tc.tile_pool(name="pa", bufs=1, space="PSUM") as pa,
                tc.tile_pool(name="ps", bufs=2, space="PSUM") as ps,
                tc.tile_pool(name="po", bufs=1, space="PSUM") as po,
                tc.tile_pool(name="pq", bufs=1, space="PSUM") as pq,
                tc.tile_pool(name="xk", bufs=2) as xkp,
                tc.tile_pool(name="rtmp", bufs=2) as rtmp,
                tc.tile_pool(name="pp", bufs=3) as pp,
                tc.tile_pool(name="att", bufs=3) as att,
                tc.tile_pool(name="otp", bufs=4) as otp,
                tc.tile_pool(name="ntmp", bufs=2) as ntmp,
            ):
                fills = deque()

                def fill(n):
                    for _ in range(n):
                        if fills:
                            fills.popleft()()

                # ---------- projection emission (per chunk ch, fp8 DR) ----
                # passes: qA (heads 0,1), qB (heads 2,3), kv (k rows 0:64,
                # v rows 64:128); one rotating PSUM bank (pa, bufs=1).
                def proj_closures(ch):
                    b = ch // NJ
                    tcol = ch * CH
                    kcol = CH * (ch % NJ)
                    state = {}

                    def dma_x():
                        xk = xkp.tile([P, KT, CH], BF16, tag="xk")
                        state["xk"] = xk
                        for k in range(KT):
                            nc.sync.dma_start(
                                xk[:, k, :],
                                xT_d[k * P:(k + 1) * P, tcol:tcol + CH])

                    def mk_mm(wsb, wlo, whi, key, k0, k1):
                        def go():
                            if key not in state:
                                state[key] = pa.tile([P, CH], F32, tag="proj", name="proj_" + key)
                            pt = state[key]
                            xk = state["xk"]
                            for k in range(k0, k1):
                                nc.tensor.matmul(
                                    pt[:], wsb[:, k, wlo:whi], xk[:, k, :],
                                    start=(k == 0), stop=(k == KT - 1))
                        return go

                    def rope_q(key, dst):
                        # dst = src*cos + shuffle(src)*sn' (sign baked in sn)
                        def go():
                            pt = state.pop(key)
                            csl = cs_sb[:, kcol:kcol + CH]
                            snl = sn_sb[:, kcol:kcol + CH]
                            qs = rtmp.tile([P, CH], BF16, tag="qs")
                            qsw = rtmp.tile([P, CH], BF16, tag="qsw")
                            nc.vector.tensor_copy(qs[:], pt[:])
                            nc.vector.stream_shuffle(qsw[:], qs[:], SWAP)
                            t2 = rtmp.tile([P, CH], BF16, tag="t2")
                            nc.gpsimd.tensor_mul(t2[:], qs[:], csl)
                            nc.vector.tensor_mul(qsw[:], qsw[:], snl)
                            nc.vector.tensor_add(dst, t2[:], qsw[:])
                        return go

                    def rope_kv():
                        pt = state.pop("kv")
                        csl = cs_sb[0:HD, kcol:kcol + CH]
                        snl = sn_sb[0:HD, kcol:kcol + CH]
                        ks = rtmp.tile([HD, CH], BF16, tag="ks")
                        ksw = rtmp.tile([HD, CH], BF16, tag="ksw")
                        nc.vector.tensor_copy(ks[:], pt[0:HD, :])
                        nc.vector.stream_shuffle(ksw[:], ks[:], SWAP)
                        kdst = k_sb[0:HD, b, kcol:kcol + CH]
                        t2k = rtmp.tile([HD, CH], BF16, tag="t2k")
                        nc.gpsimd.tensor_mul(t2k[:], ks[:], csl)
                        nc.vector.tensor_mul(ksw[:], ksw[:], snl)
                        nc.vector.tensor_add(kdst, t2k[:], ksw[:])
                        # duplicate k rows to partitions 64:128 (scores row-tiling)
                        nc.gpsimd.tensor_copy(k_sb[HD:P, b, kcol:kcol + CH], kdst)
                        # stage v rows (f32) for PE transpose
                        vs = rtmp.tile([HD, CH], F32, tag="vs")
                        nc.scalar.copy(vs[:], pt[HD:P, :])
                        state["vs"] = vs

                    def vtrans():
                        vs = state.pop("vs")
                        vt = pa.tile([P, CH], F32, tag="proj", name="proj_vt")
                        for tt in range(CH // P):
                            nc.tensor.transpose(
                                vt[:, tt * HD:(tt + 1) * HD],
                                vs[:, tt * P:(tt + 1) * P], id64_sb[:])
                        t0 = b * NI + (ch % NJ) * (CH // P)
                        nc.vector.tensor_copy(
                            v_sb[:, t0:t0 + CH // P, 0:HD],
                            vt[:, 0:(CH // P) * HD]
                            .rearrange("p (tt d) -> p tt d", tt=CH // P))

                    qa_dst = q_sb[:, 0, tcol:tcol + CH]
                    qb_dst = q_sb[:, 1, tcol:tcol + CH]
                    return [
                        dma_x,
                        mk_mm(wq_sb, 0, P, "qa", 0, 6),
                        mk_mm(wq_sb, 0, P, "qa", 6, 11),
                        mk_mm(wq_sb, 0, P, "qa", 11, KT),
                        rope_q("qa", qa_dst),
                        mk_mm(wq_sb, P, 256, "qb", 0, 6),
                        mk_mm(wq_sb, P, 256, "qb", 6, 11),
                        mk_mm(wq_sb, P, 256, "qb", 11, KT),
                        rope_q("qb", qb_dst),
                        mk_mm(wkv_sb, 0, P, "kv", 0, 6),
                        mk_mm(wkv_sb, 0, P, "kv", 6, 11),
                        mk_mm(wkv_sb, 0, P, "kv", 11, KT),
                        rope_kv,
                        vtrans,
                    ]

                # ---------- out-projection unit -----------------------
                cast_flip = [0]

                def emit_outproj(og_p, tcol_p, tt, dc):
                    op = pq.tile([P, CH], F32, tag="pout", name="pout_u")
                    for ft in range(2):
                        nc.tensor.matmul(
                            op[:],
                            og_p[:, ft, tt * P:(tt + 1) * P],
                            wo_sb[:, ft, dc * CH:(dc + 1) * CH],
                            start=(ft == 0), stop=(ft == 1))
                    ob = otp.tile([P, CH], BF16, tag="ob")
                    cast_flip[0] ^= 1
                    if cast_flip[0]:
                        nc.vector.tensor_copy(ob[:], op[:])
                    else:
                        nc.scalar.copy(ob[:], op[:])
                    nc.sync.dma_start(
                        out_d[tcol_p + tt * P:tcol_p + (tt + 1) * P,
                              dc * CH:(dc + 1) * CH],
                        ob[:])

                # ---------- prologue: first chunk's projection ---------
                for cl in proj_closures(0):
                    cl()
                fills.extend(proj_closures(1))

                # ---------- merged attention loop ----------------------
                for s in range(NCH):
                    b, j = s // NJ, s % NJ
                    tcol = s * CH
                    imax = (CH // P) * j + (CH // P) - 1
                    # safety: og ring is 3 deep; don't let the queue lag
                    # more than ~1.5 chunks of work
                    while len(fills) > 60:
                        fills.popleft()()
                    if s + 1 < NCH:
                        fills.extend(proj_closures(s + 1))
                    og_c = att.tile([P, 2, CH], BF16, tag="og")
                    iters_total = 2 * (imax + 1)
                    it = 0
                    for hp in range(2):
                        ot_acc = [po.tile([HD + 1, CH], F32, tag=f"po{h}",
                                          name=f"po_{s}_{hp}_{h}")
                                  for h in (0, 1)]
                        for i in range(imax + 1):
                            c0 = max(0, P * i - CH * j)
                            diag = P * i >= CH * j
                            sp2 = ps.tile([P, 2, CH], F32, tag="ps")
                            for hi, h in enumerate((2 * hp, 2 * hp + 1)):
                                hb = HD * (h % 2)
                                ht = h // 2
                                nc.tensor.matmul(
                                    sp2[:, hi, c0:CH],
                                    k_sb[hb:hb + HD, b, P * i:P * (i + 1)],
                                    q_sb[hb:hb + HD, ht, tcol + c0:tcol + CH],
                                    start=True, stop=True,
                                    skip_group_check=True)
                            if diag:
                                nc.vector.tensor_add(
                                    sp2[:, :, c0:c0 + P],
                                    sp2[:, :, c0:c0 + P],
                                    dmask_sb[:])
                            pt2 = pp.tile([P, 2, CH], BF16, tag="pt")
                            nc.scalar.activation(
                                pt2[:, :, c0:CH], sp2[:, :, c0:CH],
                                mybir.ActivationFunctionType.Exp,
                                scale=0.125)
                            for hi in range(2):
                                nc.tensor.matmul(
                                    ot_acc[hi][:, c0:CH],
                                    v_sb[:, b * NI + i, :],
                                    pt2[:, hi, c0:CH],
                                    start=(i == 0), stop=(i == imax),
                                    skip_group_check=True)
                            it += 1
                            left = iters_total - it + 2
                            n = -(-len(fills) // max(1, left))
                            fill(max(1, min(n, 6)))
                        # normalization for the group's 2 heads
                        for hi, h in enumerate((2 * hp, 2 * hp + 1)):
                            hb = HD * (h % 2)
                            ht = h // 2
                            otr = ntmp.tile([1, CH], F32, tag=f"otr{hi}",
                                            name=f"otr{hi}")
                            dr1 = ntmp.tile([1, CH], F32, tag=f"dr{hi}",
                                            name=f"dr{hi}")
                            nc.vector.tensor_copy(
                                otr[:], ot_acc[hi][HD:HD + 1, :])
                            nc.vector.reciprocal_approx_fast(dr1[:], otr[:])
                            lr = ntmp.tile([HD, CH], F32, tag=f"lr{hi}")
                            nc.gpsimd.partition_broadcast(lr[:], dr1[:])
                            nc.vector.tensor_mul(
                                og_c[hb:hb + HD, ht, :],
                                ot_acc[hi][0:HD, :], lr[:])
                        fill(3)
                    fills.extend(
                        (lambda a=og_c, t=tcol, tt=tt, dc=dc:
                         emit_outproj(a, t, tt, dc))
                        for tt in range(CH // P) for dc in range(D // CH))
                while fills:
                    fills.popleft()()
    nc.compile()
    return nc


def host_prep(x, wq, wk, wv, wo, cos, sin, core, B=2, T=2048, D=2048):
    """Per-core input map. Core c owns KV head c and Q heads 4c..4c+3."""
    BT = B * T
    xT = np.ascontiguousarray(x.reshape(BT, D).T.astype(NPBF16))
    wqT = np.ascontiguousarray(
        wq[256 * core:256 * (core + 1)].T.astype(NPBF16))
    wkvT = np.ascontiguousarray(np.concatenate(
        [wk[HD * core:HD * (core + 1)],
         wv[HD * core:HD * (core + 1)]], axis=0).T.astype(NPBF16))
    woT = np.ascontiguousarray(
        wo[:, 256 * core:256 * (core + 1)].T.astype(NPBF16))
    idx = (np.arange(P) % HD) // 2
    cs = np.ascontiguousarray(cos[:T, idx].T.astype(NPBF16))
    sgn = np.where(np.arange(P) % 2 == 0, -1.0, 1.0)[:, None]
    sn = np.ascontiguousarray((sin[:T, idx].T * sgn).astype(NPBF16))
    dmask = np.where(np.arange(P)[:, None] <= np.arange(P)[None, :],
                     0.0, -1e10).astype(np.float32)
    return {
        "xT": xT, "wqT": wqT, "wkvT": wkvT, "woT": woT,
        "cs": cs, "sn": sn, "dmask": dmask,
        "id64": np.eye(HD, dtype=np.float32),
    }


_CACHE = {}


def _get_program(B, T, D):
    key = (B, T, D)
    if key not in _CACHE:
        _CACHE[key] = build_program(B, T, D)
    return _CACHE[key]


def run_on_hw(x, wq, wk, wv, wo, cos, sin, B=2, T=2048, D=2048, trace=False, **kw):
    nc = _get_program(B, T, D)
    in_maps = [host_prep(x, wq, wk, wv, wo, cos, sin, c, B, T, D)
               for c in range(NCORES)]
    res = run_bass_kernel_spmd(nc, in_maps, list(range(NCORES)), trace=trace, **kw)
    parts = [np.asarray(r["out"], dtype=np.float32) for r in res.results]
    out = sum(parts).astype(np.float32).reshape(B, T, D)
    return out, res


def kernel(x, mask, wq, wk, wv, wo, cos, sin):
    out, _ = run_on_hw(np.asarray(x, np.float32), np.asarray(wq, np.float32),
                       np.asarray(wk, np.float32), np.asarray(wv, np.float32),
                       np.asarray(wo, np.float32), np.asarray(cos, np.float32),
                       np.asarray(sin, np.float32))
    return out


# revision 15
# speedup vs baseline: 1.2585x; 1.2585x over previous
"""Bass/Tile TRN2 kernel for GroupedQueryAttention (B=2, T=2048, D=2048,
32 Q heads / 8 KV heads, hd=64, RoPE, causal), sharded over 8 NeuronCores
by KV head (1 KV head + 4 Q heads per core; wo row-sharded, partials
summed on host).

v4 (bf16 everywhere): RoPE pair-swap via DVE stream_shuffle (sign baked
into the sin table; no PE permutation matmuls). Projections for chunks
0..4 run as a dense dedicated phase (deep PSUM pipelining, ACT idle so
the v-stage copy never stalls the PE). Attention processes chunks in
order; projections for chunks 5..7 and all out-projection units are
woven into the attention i-loop as spread-out fill work. attV matmuls
are emitted with a one-iteration lag so the PE queue head never waits
on the exp (ACT) producer. Scores K=64 head-pairs run concurrently via
PE row tiling (partitions 0:64 / 64:128)."""

import sys

for _p in ("/opt/trn_rl_repo",):
    if _p not in sys.path:
        sys.path.insert(0, _p)

from collections import deque

import numpy as np
import ml_dtypes

import concourse.bass as bass
import concourse.mybir as mybir
import concourse.tile as tile
from concourse import bacc
from concourse.bass_utils import run_bass_kernel_spmd

F32 = mybir.dt.float32
BF16 = mybir.dt.bfloat16
NPBF16 = ml_dtypes.bfloat16
P = 128
HD = 64          # head dim
CH = 512         # token chunk
NCORES = 8
NPRE = 5         # chunks projected in the dedicated phase


def build_program(B=2, T=2048, D=2048):
    BT = B * T
    KT = D // P            # contraction tiles for projections (16)
    NCH = BT // CH         # chunks over all batches (8)
    NJ = T // CH           # q chunks per batch (4)
    NI = T // P            # k tiles per batch (16)

    nc = bacc.Bacc(None, target_bir_lowering=False, debug=False)

    xT_d = nc.dram_tensor("xT", [D, BT], BF16, kind="ExternalInput")
    wq_d = nc.dram_tensor("wqT", [D, 256], BF16, kind="ExternalInput")
    wkv_d = nc.dram_tensor("wkvT", [D, 128], BF16, kind="ExternalInput")
    wo_d = nc.dram_tensor("woT", [256, D], BF16, kind="ExternalInput")
    cs_d = nc.dram_tensor("cs", [P, T], BF16, kind="ExternalInput")
    sn_d = nc.dram_tensor("sn", [P, T], BF16, kind="ExternalInput")
    dmask_d = nc.dram_tensor("dmask", [P, P], F32, kind="ExternalInput")
    id64_d = nc.dram_tensor("id64", [HD, HD], F32, kind="ExternalInput")
    out_d = nc.dram_tensor("out", [BT, D], BF16, kind="ExternalOutput")

    SWAP = [(i ^ 1) for i in range(32)]

    with tile.TileContext(nc) as tc:
        with tc.tile_pool(name="persist", bufs=1) as persist:
            wq_sb = persist.tile([P, KT, 256], BF16, tag="wq")
            wkv_sb = persist.tile([P, KT, 128], BF16, tag="wkv")
            wo_sb = persist.tile([P, 2, D], BF16, tag="wo")
            cs_sb = persist.tile([P, T], BF16, tag="cs")
            sn_sb = persist.tile([P, T], BF16, tag="sn")
            dmask_sb = persist.tile([P, 2, P], F32, tag="dmask")
            id64_sb = persist.tile([HD, HD], F32, tag="id64")
            q_sb = persist.tile([P, 2, BT], BF16, tag="qcache")
            k_sb = persist.tile([P, B, T], BF16, tag="kcache")
            v_sb = persist.tile([P, B * NI, HD + 1], BF16, tag="vcache")

            nc.sync.dma_start(wq_sb[:], wq_d[:].rearrange("(ko p) m -> p ko m", p=P))
            nc.sync.dma_start(wkv_sb[:], wkv_d[:].rearrange("(ko p) m -> p ko m", p=P))
            nc.sync.dma_start(wo_sb[:], wo_d[:].rearrange("(fo p) n -> p fo n", p=P))
            nc.sync.dma_start(cs_sb[:], cs_d[:])
            nc.sync.dma_start(sn_sb[:], sn_d[:])
            nc.sync.dma_start(dmask_sb[:, 0, :], dmask_d[:])
            nc.sync.dma_start(dmask_sb[:, 1, :], dmask_d[:])
            nc.sync.dma_start(id64_sb[:], id64_d[:])
            nc.vector.memset(v_sb[:, :, HD:HD + 1], 1.0)

            with (
                tc.tile_pool(name="xk", bufs=3) as xkp,
                tc.tile_pool(name="rtmp", bufs=3) as rtmp,
            ):
                # ---------- shared projection pieces -------------------
                def emit_dma_x(state, ch):
                    xk = xkp.tile([P, KT, CH], BF16, tag="xk")
                    state["xk"] = xk
                    for k in range(KT):
                        nc.sync.dma_start(
                            xk[:, k, :],
                            xT_d[k * P:(k + 1) * P, ch * CH:(ch + 1) * CH])

                def emit_mm(pool, state, wsb, wlo, whi, key, k0, k1):
                    if key not in state:
                        state[key] = pool.tile([P, CH], F32, tag="proj",
                                               name="proj_" + key)
                    pt = state[key]
                    xk = state["xk"]
                    for k in range(k0, k1):
                        nc.tensor.matmul(
                            pt[:], wsb[:, k, wlo:whi], xk[:, k, :],
                            start=(k == 0), stop=(k == KT - 1))

                def emit_rope(state, key, dst, kcol, vstage_dve):
                    # dst = src*cos + shuffle(src)*sn' (sign baked in sn)
                    pt = state.pop(key)
                    rows = P if key != "kv" else HD
                    csl = cs_sb[0:rows, kcol:kcol + CH]
                    snl = sn_sb[0:rows, kcol:kcol + CH]
                    qs = rtmp.tile([P, CH], BF16, tag="qs", name="qs")
                    qsw = rtmp.tile([P, CH], BF16, tag="qsw", name="qsw")
                    nc.vector.tensor_copy(qs[0:rows, :], pt[0:rows, :])
                    nc.vector.stream_shuffle(qsw[0:rows, :], qs[0:rows, :], SWAP)
                    t2 = rtmp.tile([P, CH], BF16, tag="t2", name="t2")
                    nc.gpsimd.tensor_mul(t2[0:rows, :], qs[0:rows, :], csl)
                    nc.vector.tensor_mul(qsw[0:rows, :], qsw[0:rows, :], snl)
                    nc.vector.tensor_add(dst, t2[0:rows, :], qsw[0:rows, :])
                    if key == "kv":
                        # stage v rows for the PE transpose
                        vs = rtmp.tile([HD, CH], F32, tag="vs", name="vs")
                        if vstage_dve:
                            nc.vector.tensor_copy(vs[:], pt[HD:P, :])
                        else:
                            nc.scalar.copy(vs[:], pt[HD:P, :])
                        state["vs"] = vs

                def emit_kdup(b, kcol):
                    nc.gpsimd.tensor_copy(k_sb[HD:P, b, kcol:kcol + CH],
                                          k_sb[0:HD, b, kcol:kcol + CH])

                def emit_vtrans(pool, state, b, ch):
                    vs = state.pop("vs")
                    vt = pool.tile([P, CH], F32, tag="proj", name="proj_vt")
                    for tt in range(CH // P):
                        nc.tensor.transpose(
                            vt[:, tt * HD:(tt + 1) * HD],
                            vs[:, tt * P:(tt + 1) * P], id64_sb[:])
                    t0 = b * NI + (ch % NJ) * (CH // P)
                    nc.vector.tensor_copy(
                        v_sb[:, t0:t0 + CH // P, 0:HD],
                        vt[:, 0:(CH // P) * HD]
                        .rearrange("p (tt d) -> p tt d", tt=CH // P))

                # ---------- phase A: proj chunks 0..NPRE-1 -------------
                with tc.tile_pool(name="pa", bufs=3, space="PSUM") as pa:
                    for ch in range(NPRE):
                        b = ch // NJ
                        kcol = CH * (ch % NJ)
                        state = {}
                        emit_dma_x(state, ch)
                        emit_mm(pa, state, wq_sb, 0, P, "qa", 0, KT)
                        emit_rope(state, "qa",
                                  q_sb[:, 0, ch * CH:(ch + 1) * CH], kcol, False)
                        emit_mm(pa, state, wq_sb, P, 256, "qb", 0, KT)
                        emit_rope(state, "qb",
                                  q_sb[:, 1, ch * CH:(ch + 1) * CH], kcol, False)
                        emit_mm(pa, state, wkv_sb, 0, P, "kv", 0, KT)
                        emit_rope(state, "kv",
                                  k_sb[0:HD, b, kcol:kcol + CH], kcol, False)
                        emit_kdup(b, kcol)
                        emit_vtrans(pa, state, b, ch)

                # ---------- phase B: attention + woven fills -----------
                with (
                    tc.tile_pool(name="pb", bufs=1, space="PSUM") as pb,
                    tc.tile_pool(name="ps", bufs=2, space="PSUM") as ps,
                    tc.tile_pool(name="po", bufs=1, space="PSUM") as po,
                    tc.tile_pool(name="pq", bufs=1, space="PSUM") as pq,
                    tc.tile_pool(name="pp", bufs=3) as pp,
                    tc.tile_pool(name="att", bufs=3) as att,
                    tc.tile_pool(name="otp", bufs=4) as otp,
                    tc.tile_pool(name="ntmp", bufs=2) as ntmp,
                ):
                    fills = deque()

                    def fill(n):
                        for _ in range(n):
                            if fills:
                                fills.popleft()()

                    def proj_closures(ch):
                        b = ch // NJ
                        kcol = CH * (ch % NJ)
                        state = {}
                        nop = lambda: None
                        return [
                            lambda: emit_dma_x(state, ch),
                            lambda: emit_mm(pb, state, wq_sb, 0, P, "qa", 0, 6),
                            lambda: emit_mm(pb, state, wq_sb, 0, P, "qa", 6, 11),
                            lambda: emit_mm(pb, state, wq_sb, 0, P, "qa", 11, KT),
                            lambda: emit_rope(
                                state, "qa",
                                q_sb[:, 0, ch * CH:(ch + 1) * CH], kcol, True),
                            lambda: emit_mm(pb, state, wq_sb, P, 256, "qb", 0, 6),
                            lambda: emit_mm(pb, state, wq_sb, P, 256, "qb", 6, 11),
                            lambda: emit_mm(pb, state, wq_sb, P, 256, "qb", 11, KT),
                            lambda: emit_rope(
                                state, "qb",
                                q_sb[:, 1, ch * CH:(ch + 1) * CH], kcol, True),
                            lambda: emit_mm(pb, state, wkv_sb, 0, P, "kv", 0, 6),
                            lambda: emit_mm(pb, state, wkv_sb, 0, P, "kv", 6, 11),
                            lambda: emit_mm(pb, state, wkv_sb, 0, P, "kv", 11, KT),
                            lambda: emit_rope(
                                state, "kv",
                                k_sb[0:HD, b, kcol:kcol + CH], kcol, True),
                            lambda: emit_kdup(b, kcol),
                            nop, nop, nop,
                            lambda: emit_vtrans(pb, state, b, ch),
                        ]

                    cast_flip = [0]

                    def emit_outproj(og_p, tcol_p, tt, dc):
                        op = pq.tile([P, CH], F32, tag="pout", name="pout_u")
                        for ft in range(2):
                            nc.tensor.matmul(
                                op[:],
                                og_p[:, ft, tt * P:(tt + 1) * P],
                                wo_sb[:, ft, dc * CH:(dc + 1) * CH],
                                start=(ft == 0), stop=(ft == 1))
                        ob = otp.tile([P, CH], BF16, tag="ob")
                        cast_flip[0] ^= 1
                        if cast_flip[0]:
                            nc.vector.tensor_copy(ob[:], op[:])
                        else:
                            nc.scalar.copy(ob[:], op[:])
                        nc.sync.dma_start(
                            out_d[tcol_p + tt * P:tcol_p + (tt + 1) * P,
                                  dc * CH:(dc + 1) * CH],
                            ob[:])

                    for s in range(NPRE, NCH):
                        fills.extend(proj_closures(s))

                    for s in range(NCH):
                        b, j = s // NJ, s % NJ
                        tcol = s * CH
                        imax = (CH // P) * j + (CH // P) - 1
                        og_c = att.tile([P, 2, CH], BF16, tag="og")
                        iters_total = 2 * (imax + 1)
                        it = 0
                        for hp in range(2):
                            ot_acc = [po.tile([HD + 1, CH], F32, tag=f"po{h}",
                                              name=f"po_{s}_{hp}_{h}")
                                      for h in (0, 1)]
                            lag = None
                            for i in range(imax + 1):
                                c0 = max(0, P * i - CH * j)
                                diag = P * i >= CH * j
                                sp2 = ps.tile([P, 2, CH], F32, tag="ps",
                                              name="sp2")
                                for hi, h in enumerate((2 * hp, 2 * hp + 1)):
                                    hb = HD * (h % 2)
                                    ht = h // 2
                                    nc.tensor.matmul(
                                        sp2[:, hi, c0:CH],
                                        k_sb[hb:hb + HD, b, P * i:P * (i + 1)],
                                        q_sb[hb:hb + HD, ht,
                                             tcol + c0:tcol + CH],
                                        start=True, stop=True,
                                        skip_group_check=True)
                                if diag:
                                    nc.vector.tensor_add(
                                        sp2[:, :, c0:c0 + P],
                                        sp2[:, :, c0:c0 + P],
                                        dmask_sb[:])
                                fill(1)
                                pt2 = pp.tile([P, 2, CH], BF16, tag="pt")
                                nc.scalar.activation(
                                    pt2[:, :, c0:CH], sp2[:, :, c0:CH],
                                    mybir.ActivationFunctionType.Exp,
                                    scale=0.125)
                                if lag is not None:
                                    lpt, li, lc0 = lag
                                    for hi in range(2):
                                        nc.tensor.matmul(
                                            ot_acc[hi][:, lc0:CH],
                                            v_sb[:, b * NI + li, :],
                                            lpt[:, hi, lc0:CH],
                                            start=(li == 0),
                                            stop=(li == imax),
                                            skip_group_check=True)
                                lag = (pt2, i, c0)
                                it += 1
                                left = iters_total - it + 2
                                n = -(-len(fills) // max(1, left))
                                fill(max(1, min(n, 3)))
                            # drain the lagged attV
                            lpt, li, lc0 = lag
                            for hi in range(2):
                                nc.tensor.matmul(
                                    ot_acc[hi][:, lc0:CH],
                                    v_sb[:, b * NI + li, :],
                                    lpt[:, hi, lc0:CH],
                                    start=(li == 0), stop=(li == imax),
                                    skip_group_check=True)
                            # normalization for the group's 2 heads
                            for hi, h in enumerate((2 * hp, 2 * hp + 1)):
                                hb = HD * (h % 2)
                                ht = h // 2
                                otr = ntmp.tile([1, CH], F32, tag=f"otr{hi}",
                                                name=f"otr{hi}")
                                dr1 = ntmp.tile([1, CH], F32, tag=f"dr{hi}",
                                                name=f"dr{hi}")
                                nc.vector.tensor_copy(
                                    otr[:], ot_acc[hi][HD:HD + 1, :])
                                nc.vector.reciprocal_approx_fast(dr1[:], otr[:])
                                lr = ntmp.tile([HD, CH], F32, tag=f"lr{hi}")
                                nc.gpsimd.partition_broadcast(lr[:], dr1[:])
                                nc.vector.tensor_mul(
                                    og_c[hb:hb + HD, ht, :],
                                    ot_acc[hi][0:HD, :], lr[:])
                                fill(1)
                            fill(2)
                        fills.extend(
                            (lambda a=og_c, t=tcol, tt=tt, dc=dc:
                             emit_outproj(a, t, tt, dc))
                            for tt in range(CH // P) for dc in range(D // CH))
                    while fills:
                        fills.popleft()()
    nc.compile()
    return nc


def host_prep(x, wq, wk, wv, wo, cos, sin, core, B=2, T=2048, D=2048):
    """Per-core input map. Core c owns KV head c and Q heads 4c..4c+3."""
    BT = B * T
    xT = np.ascontiguousarray(x.reshape(BT, D).T.astype(NPBF16))
    wqT = np.ascontiguousarray(
        wq[256 * core:256 * (core + 1)].T.astype(NPBF16))
    wkvT = np.ascontiguousarray(np.concatenate(
        [wk[HD * core:HD * (core + 1)],
         wv[HD * core:HD * (core + 1)]], axis=0).T.astype(NPBF16))
    woT = np.ascontiguousarray(
        wo[:, 256 * core:256 * (core + 1)].T.astype(NPBF16))
    idx = (np.arange(P) % HD) // 2
    cs = np.ascontiguousarray(cos[:T, idx].T.astype(NPBF16))
    sgn = np.where(np.arange(P) % 2 == 0, -1.0, 1.0)[:, None]
    sn = np.ascontiguousarray((sin[:T, idx].T * sgn).astype(NPBF16))
    dmask = np.where(np.arange(P)[:, None] <= np.arange(P)[None, :],
                     0.0, -1e10).astype(np.float32)
    return {
        "xT": xT, "wqT": wqT, "wkvT": wkvT, "woT": woT,
        "cs": cs, "sn": sn, "dmask": dmask,
        "id64": np.eye(HD, dtype=np.float32),
    }


_CACHE = {}


def _get_program(B, T, D):
    key = (B, T, D)
    if key not in _CACHE:
        _CACHE[key] = build_program(B, T, D)
    return _CACHE[key]


def run_on_hw(x, wq, wk, wv, wo, cos, sin, B=2, T=2048, D=2048, trace=False, **kw):
    nc = _get_program(B, T, D)
    in_maps = [host_prep(x, wq, wk, wv, wo, cos, sin, c, B, T, D)
               for c in range(NCORES)]
    res = run_bass_kernel_spmd(nc, in_maps, list(range(NCORES)), trace=trace, **kw)
    parts = [np.asarray(r["out"], dtype=np.float32) for r in res.results]
    out = sum(parts).astype(np.float32).reshape(B, T, D)
    return out, res


def kernel(x, mask, wq, wk, wv, wo, cos, sin):
    out, _ = run_on_hw(np.asarray(x, np.float32), np.asarray(wq, np.float32),
                       np.asarray(wk, np.float32), np.asarray(wv, np.float32),
                       np.asarray(wo, np.float32), np.asarray(cos, np.float32),
                       np.asarray(sin, np.float32))
    return out


# revision 16
# speedup vs baseline: 1.3912x; 1.1054x over previous
"""Bass/Tile TRN2 kernel for GroupedQueryAttention (B=2, T=2048, D=2048,
32 Q heads / 8 KV heads, hd=64, RoPE, causal), sharded over 8 NeuronCores
by KV head (1 KV head + 4 Q heads per core; wo row-sharded, partials
summed on host).

v5 (bf16): phase A projects all 8 chunks densely (pa bufs=3, v-transposes
deferred one chunk so the ACT-staged copy never blocks the PE, x tiles
DMA-prefetched two chunks ahead). RoPE pair-swap via DVE stream_shuffle
(sign baked into the sin table; no PE permutation matmuls). Phase B runs
attention with out-projection units woven in as fill work (pq bufs=2 so
fills aren't cast-gated), attV emitted with a one-iteration lag so the
PE queue never waits on exp (ACT), and causal masking applied as a cheap
bf16 triangle multiply on the probabilities instead of a f32 PSUM add.
Scores K=64 head-pairs run concurrently via PE row tiling."""

import sys

for _p in ("/opt/trn_rl_repo",):
    if _p not in sys.path:
        sys.path.insert(0, _p)

from collections import deque

import numpy as np
import ml_dtypes

import concourse.bass as bass
import concourse.mybir as mybir
import concourse.tile as tile
from concourse import bacc
from concourse.bass_utils import run_bass_kernel_spmd

F32 = mybir.dt.float32
BF16 = mybir.dt.bfloat16
NPBF16 = ml_dtypes.bfloat16
P = 128
HD = 64          # head dim
CH = 512         # token chunk
NCORES = 8


def build_program(B=2, T=2048, D=2048):
    BT = B * T
    KT = D // P            # contraction tiles for projections (16)
    NCH = BT // CH         # chunks over all batches (8)
    NJ = T // CH           # q chunks per batch (4)
    NI = T // P            # k tiles per batch (16)

    nc = bacc.Bacc(None, target_bir_lowering=False, debug=False)

    xT_d = nc.dram_tensor("xT", [D, BT], BF16, kind="ExternalInput")
    wq_d = nc.dram_tensor("wqT", [D, 256], BF16, kind="ExternalInput")
    wkv_d = nc.dram_tensor("wkvT", [D, 128], BF16, kind="ExternalInput")
    wo_d = nc.dram_tensor("woT", [256, D], BF16, kind="ExternalInput")
    cs_d = nc.dram_tensor("cs", [P, T], BF16, kind="ExternalInput")
    sn_d = nc.dram_tensor("sn", [P, T], BF16, kind="ExternalInput")
    tri_d = nc.dram_tensor("tri", [P, P], BF16, kind="ExternalInput")
    id64_d = nc.dram_tensor("id64", [HD, HD], F32, kind="ExternalInput")
    out_d = nc.dram_tensor("out", [BT, D], BF16, kind="ExternalOutput")

    SWAP = [(i ^ 1) for i in range(32)]

    with tile.TileContext(nc) as tc:
        with tc.tile_pool(name="persist", bufs=1) as persist:
            wq_sb = persist.tile([P, KT, 256], BF16, tag="wq")
            wkv_sb = persist.tile([P, KT, 128], BF16, tag="wkv")
            wo_sb = persist.tile([P, 2, D], BF16, tag="wo")
            cs_sb = persist.tile([P, T], BF16, tag="cs")
            sn_sb = persist.tile([P, T], BF16, tag="sn")
            tri_sb = persist.tile([P, 2, P], BF16, tag="tri")
            id64_sb = persist.tile([HD, HD], F32, tag="id64")
            q_sb = persist.tile([P, 2, BT], BF16, tag="qcache")
            k_sb = persist.tile([P, B, T], BF16, tag="kcache")
            v_sb = persist.tile([P, B * NI, HD + 1], BF16, tag="vcache")

            nc.sync.dma_start(wq_sb[:], wq_d[:].rearrange("(ko p) m -> p ko m", p=P))
            nc.sync.dma_start(wkv_sb[:], wkv_d[:].rearrange("(ko p) m -> p ko m", p=P))
            nc.sync.dma_start(wo_sb[:], wo_d[:].rearrange("(fo p) n -> p fo n", p=P))
            nc.sync.dma_start(cs_sb[:], cs_d[:])
            nc.sync.dma_start(sn_sb[:], sn_d[:])
            nc.sync.dma_start(tri_sb[:, 0, :], tri_d[:])
            nc.sync.dma_start(tri_sb[:, 1, :], tri_d[:])
            nc.sync.dma_start(id64_sb[:], id64_d[:])
            nc.vector.memset(v_sb[:, :, HD:HD + 1], 1.0)

            with (
                tc.tile_pool(name="xk", bufs=3) as xkp,
                tc.tile_pool(name="rtmp", bufs=3) as rtmp,
            ):
                # ---------- shared projection pieces -------------------
                def emit_dma_x(state, ch):
                    xk = xkp.tile([P, KT, CH], BF16, tag="xk")
                    state["xk"] = xk
                    for k in range(KT):
                        nc.sync.dma_start(
                            xk[:, k, :],
                            xT_d[k * P:(k + 1) * P, ch * CH:(ch + 1) * CH])

                def emit_mm(pool, state, wsb, wlo, whi, key, k0, k1):
                    if key not in state:
                        state[key] = pool.tile([P, CH], F32, tag="proj",
                                               name="proj_" + key)
                    pt = state[key]
                    xk = state["xk"]
                    for k in range(k0, k1):
                        nc.tensor.matmul(
                            pt[:], wsb[:, k, wlo:whi], xk[:, k, :],
                            start=(k == 0), stop=(k == KT - 1))

                def emit_rope(state, key, dst, kcol):
                    # dst = src*cos + shuffle(src)*sn' (sign baked in sn)
                    pt = state.pop(key)
                    rows = P if key != "kv" else HD
                    csl = cs_sb[0:rows, kcol:kcol + CH]
                    snl = sn_sb[0:rows, kcol:kcol + CH]
                    qs = rtmp.tile([P, CH], BF16, tag="qs", name="qs")
                    qsw = rtmp.tile([P, CH], BF16, tag="qsw", name="qsw")
                    nc.vector.tensor_copy(qs[0:rows, :], pt[0:rows, :])
                    nc.vector.stream_shuffle(qsw[0:rows, :], qs[0:rows, :], SWAP)
                    t2 = rtmp.tile([P, CH], BF16, tag="t2", name="t2")
                    nc.gpsimd.tensor_mul(t2[0:rows, :], qs[0:rows, :], csl)
                    nc.vector.tensor_mul(qsw[0:rows, :], qsw[0:rows, :], snl)
                    nc.vector.tensor_add(dst, t2[0:rows, :], qsw[0:rows, :])
                    if key == "kv":
                        # stage v rows for the (deferred) PE transpose
                        vs = rtmp.tile([HD, CH], F32, tag="vs", name="vs")
                        nc.scalar.copy(vs[:], pt[HD:P, :])
                        state["vs"] = vs

                def emit_kdup(b, kcol):
                    nc.gpsimd.tensor_copy(k_sb[HD:P, b, kcol:kcol + CH],
                                          k_sb[0:HD, b, kcol:kcol + CH])

                def emit_vtrans(pool, state, b, ch):
                    vs = state.pop("vs")
                    vt = pool.tile([P, CH], F32, tag="proj", name="proj_vt")
                    for tt in range(CH // P):
                        nc.tensor.transpose(
                            vt[:, tt * HD:(tt + 1) * HD],
                            vs[:, tt * P:(tt + 1) * P], id64_sb[:])
                    t0 = b * NI + (ch % NJ) * (CH // P)
                    nc.vector.tensor_copy(
                        v_sb[:, t0:t0 + CH // P, 0:HD],
                        vt[:, 0:(CH // P) * HD]
                        .rearrange("p (tt d) -> p tt d", tt=CH // P))

                # ---------- phase A: all projections -------------------
                with tc.tile_pool(name="pa", bufs=3, space="PSUM") as pa:
                    states = {}
                    for ch in range(2):
                        states[ch] = {}
                        emit_dma_x(states[ch], ch)
                    pend_vt = None
                    for ch in range(NCH):
                        b = ch // NJ
                        kcol = CH * (ch % NJ)
                        st = states[ch]
                        emit_mm(pa, st, wq_sb, 0, P, "qa", 0, KT)
                        if pend_vt is not None:
                            pend_vt()
                        if ch + 2 < NCH:
                            states[ch + 2] = {}
                            emit_dma_x(states[ch + 2], ch + 2)
                        emit_rope(st, "qa", q_sb[:, 0, ch * CH:(ch + 1) * CH],
                                  kcol)
                        emit_mm(pa, st, wq_sb, P, 256, "qb", 0, KT)
                        emit_rope(st, "qb", q_sb[:, 1, ch * CH:(ch + 1) * CH],
                                  kcol)
                        emit_mm(pa, st, wkv_sb, 0, P, "kv", 0, KT)
                        emit_rope(st, "kv", k_sb[0:HD, b, kcol:kcol + CH], kcol)
                        emit_kdup(b, kcol)
                        pend_vt = (lambda st=st, b=b, ch=ch:
                                   emit_vtrans(pa, st, b, ch))
                    pend_vt()

                # ---------- phase B: attention + outproj fills ---------
                with (
                    tc.tile_pool(name="ps", bufs=2, space="PSUM") as ps,
                    tc.tile_pool(name="po", bufs=1, space="PSUM") as po,
                    tc.tile_pool(name="pq", bufs=2, space="PSUM") as pq,
                    tc.tile_pool(name="pp", bufs=3) as pp,
                    tc.tile_pool(name="att", bufs=3) as att,
                    tc.tile_pool(name="otp", bufs=4) as otp,
                    tc.tile_pool(name="ntmp", bufs=2) as ntmp,
                ):
                    fills = deque()

                    def fill(n):
                        for _ in range(n):
                            if fills:
                                fills.popleft()()

                    cast_flip = [0]

                    def emit_outproj(og_p, tcol_p, tt, dc):
                        op = pq.tile([P, CH], F32, tag="pout", name="pout_u")
                        for ft in range(2):
                            nc.tensor.matmul(
                                op[:],
                                og_p[:, ft, tt * P:(tt + 1) * P],
                                wo_sb[:, ft, dc * CH:(dc + 1) * CH],
                                start=(ft == 0), stop=(ft == 1))
                        ob = otp.tile([P, CH], BF16, tag="ob")
                        cast_flip[0] = (cast_flip[0] + 1) % 3
                        if cast_flip[0] == 0:
                            nc.scalar.copy(ob[:], op[:])
                        else:
                            nc.vector.tensor_copy(ob[:], op[:])
                        nc.sync.dma_start(
                            out_d[tcol_p + tt * P:tcol_p + (tt + 1) * P,
                                  dc * CH:(dc + 1) * CH],
                            ob[:])

                    for s in range(NCH):
                        b, j = s // NJ, s % NJ
                        tcol = s * CH
                        imax = (CH // P) * j + (CH // P) - 1
                        og_c = att.tile([P, 2, CH], BF16, tag="og")
                        iters_total = 2 * (imax + 1)
                        it = 0
                        for hp in range(2):
                            ot_acc = [po.tile([HD + 1, CH], F32, tag=f"po{h}",
                                              name=f"po_{s}_{hp}_{h}")
                                      for h in (0, 1)]
                            lag = None
                            for i in range(imax + 1):
                                c0 = max(0, P * i - CH * j)
                                diag = P * i >= CH * j
                                sp2 = ps.tile([P, 2, CH], F32, tag="ps",
                                              name="sp2")
                                for hi, h in enumerate((2 * hp, 2 * hp + 1)):
                                    hb = HD * (h % 2)
                                    ht = h // 2
                                    nc.tensor.matmul(
                                        sp2[:, hi, c0:CH],
                                        k_sb[hb:hb + HD, b, P * i:P * (i + 1)],
                                        q_sb[hb:hb + HD, ht,
                                             tcol + c0:tcol + CH],
                                        start=True, stop=True,
                                        skip_group_check=True)
                                fill(1)
                                pt2 = pp.tile([P, 2, CH], BF16, tag="pt")
                                nc.scalar.activation(
                                    pt2[:, :, c0:CH], sp2[:, :, c0:CH],
                                    mybir.ActivationFunctionType.Exp,
                                    scale=0.125)
                                if diag:
                                    # zero the k>q triangle (bf16, 2x mode)
                                    nc.vector.tensor_mul(
                                        pt2[:, :, c0:c0 + P],
                                        pt2[:, :, c0:c0 + P],
                                        tri_sb[:])
                                if lag is not None:
                                    lpt, li, lc0 = lag
                                    for hi in range(2):
                                        nc.tensor.matmul(
                                            ot_acc[hi][:, lc0:CH],
                                            v_sb[:, b * NI + li, :],
                                            lpt[:, hi, lc0:CH],
                                            start=(li == 0),
                                            stop=(li == imax),
                                            skip_group_check=True)
                                lag = (pt2, i, c0)
                                it += 1
                                left = iters_total - it + 2
                                n = -(-max(0, len(fills) - 10) // max(1, left))
                                fill(max(1, min(n, 3)))
                            # drain the lagged attV
                            lpt, li, lc0 = lag
                            for hi in range(2):
                                nc.tensor.matmul(
                                    ot_acc[hi][:, lc0:CH],
                                    v_sb[:, b * NI + li, :],
                                    lpt[:, hi, lc0:CH],
                                    start=(li == 0), stop=(li == imax),
                                    skip_group_check=True)
                            # normalization for the group's 2 heads
                            for hi, h in enumerate((2 * hp, 2 * hp + 1)):
                                hb = HD * (h % 2)
                                ht = h // 2
                                otr = ntmp.tile([1, CH], F32, tag=f"otr{hi}",
                                                name=f"otr{hi}")
                                dr1 = ntmp.tile([1, CH], F32, tag=f"dr{hi}",
                                                name=f"dr{hi}")
                                nc.vector.tensor_copy(
                                    otr[:], ot_acc[hi][HD:HD + 1, :])
                                nc.vector.reciprocal_approx_fast(dr1[:], otr[:])
                                lr = ntmp.tile([HD, CH], F32, tag=f"lr{hi}")
                                nc.gpsimd.partition_broadcast(lr[:], dr1[:])
                                nc.vector.tensor_mul(
                                    og_c[hb:hb + HD, ht, :],
                                    ot_acc[hi][0:HD, :], lr[:])
                                fill(1)
                            fill(2)
                        fills.extend(
                            (lambda a=og_c, t=tcol, tt=tt, dc=dc:
                             emit_outproj(a, t, tt, dc))
                            for tt in range(CH // P) for dc in range(D // CH))
                    while fills:
                        fills.popleft()()
    nc.compile()
    return nc


def host_prep(x, wq, wk, wv, wo, cos, sin, core, B=2, T=2048, D=2048):
    """Per-core input map. Core c owns KV head c and Q heads 4c..4c+3."""
    BT = B * T
    xT = np.ascontiguousarray(x.reshape(BT, D).T.astype(NPBF16))
    wqT = np.ascontiguousarray(
        wq[256 * core:256 * (core + 1)].T.astype(NPBF16))
    wkvT = np.ascontiguousarray(np.concatenate(
        [wk[HD * core:HD * (core + 1)],
         wv[HD * core:HD * (core + 1)]], axis=0).T.astype(NPBF16))
    woT = np.ascontiguousarray(
        wo[:, 256 * core:256 * (core + 1)].T.astype(NPBF16))
    idx = (np.arange(P) % HD) // 2
    cs = np.ascontiguousarray(cos[:T, idx].T.astype(NPBF16))
    sgn = np.where(np.arange(P) % 2 == 0, -1.0, 1.0)[:, None]
    sn = np.ascontiguousarray((sin[:T, idx].T * sgn).astype(NPBF16))
    # tri[kt, qt] = 1 where kt <= qt (keep), 0 where kt > qt (masked)
    tri = (np.arange(P)[:, None] <= np.arange(P)[None, :]).astype(NPBF16)
    return {
        "xT": xT, "wqT": wqT, "wkvT": wkvT, "woT": woT,
        "cs": cs, "sn": sn, "tri": tri,
        "id64": np.eye(HD, dtype=np.float32),
    }


_CACHE = {}


def _get_program(B, T, D):
    key = (B, T, D)
    if key not in _CACHE:
        _CACHE[key] = build_program(B, T, D)
    return _CACHE[key]


def run_on_hw(x, wq, wk, wv, wo, cos, sin, B=2, T=2048, D=2048, trace=False, **kw):
    nc = _get_program(B, T, D)
    in_maps = [host_prep(x, wq, wk, wv, wo, cos, sin, c, B, T, D)
               for c in range(NCORES)]
    res = run_bass_kernel_spmd(nc, in_maps, list(range(NCORES)), trace=trace, **kw)
    parts = [np.asarray(r["out"], dtype=np.float32) for r in res.results]
    out = sum(parts).astype(np.float32).reshape(B, T, D)
    return out, res


def kernel(x, mask, wq, wk, wv, wo, cos, sin):
    out, _ = run_on_hw(np.asarray(x, np.float32), np.asarray(wq, np.float32),
                       np.asarray(wk, np.float32), np.asarray(wv, np.float32),
                       np.asarray(wo, np.float32), np.asarray(cos, np.float32),
                       np.asarray(sin, np.float32))
    return out
